# revision 21
# baseline (speedup 1.0000x reference)
"""EdgeAttention GNN message passing on 8 Trainium2 NeuronCores.

Strategy (edge-parallel, receiver-sorted, gather-free, half-block windows):
  - Host: sort edges by receiver node, shard NODES uniformly across the 8
    cores (each core owns a contiguous 1/8 node range); each core processes
    exactly the edges whose receiver it owns (~E/8 by symmetry). Within a
    core, receivers are grouped into blocks of 128 nodes; each block's edges
    are split into a low half (receivers 0..63) and high half (64..127),
    each padded to a fixed number of 128-edge subtiles (uniform across
    cores/blocks so the SPMD program is identical). The host materializes
    the SENDER's raw node features per edge slot (pure data movement), so
    the device never needs an indirect gather (the Q7 descriptor path was
    measured at ~10 ns/edge serialized).
  - Device phase 1: q = lrelu(own_nodes @ Wq.T) kept in SBUF.
  - Device phase 2, per <=8-subtile batch:
      kT_e     = lrelu(Wk.T^T @ nodesT_e)     (PE w/ stationary weights,
                                               512-col rhs + one ACT)
      v        = lrelu(edgesT @ Wv.T)         (PE per subtile + one ACT)
      S        = kT_e.T @ q_half              (PE per subtile, 64 cols)
      Et       = exp(S / sqrt(d))             (one ACT per batch, 64*bns)
      oh       = is_equal(rc_loc, iota64)     (one DVE TT per batch)
      P        = Et * oh                      (DVE tensor_tensor)
      out_blk[W:W+64] += P.T @ [v | 1]        (PE; col 128 = softmax denom)
    Block epilogue: out = numer * reciprocal(denom), DMA to DRAM.
  Softmax max-subtraction is skipped: logits are O(1) here and
  exp(l)/sum(exp(l)) == exp(l-m)/sum(exp(l-m)) exactly in the reals.
"""

import sys

sys.path.insert(0, "/opt/trn_rl_repo")

import numpy as np

N_CORES = 8
P = 128
WW = 64                              # receiver window width per subtile


def _cfg_from_shapes(n_nodes, n_edges, d_v, d_e, d_attn, t_lo, t_hi):
    assert d_v % P == 0 and d_e == P and d_attn == P
    npc = -(-n_nodes // (N_CORES * P)) * P          # nodes per core, mult of 128
    while (npc * N_CORES) % 512:
        npc += P
    nb = npc // P                                   # blocks per core
    n_pad = npc * N_CORES
    t_b = t_lo + t_hi
    ntiles = nb * t_b
    # batches: per-block chunks of <=8 subtiles (never cross a block)
    chunks = []
    left = t_b
    while left > 0:
        c = min(8, left)
        if left - c == 1:
            c -= 1
        chunks.append(c)
        left -= c
    batches = []                     # (start_subtile, n_subtiles)
    for b in range(nb):
        t0 = b * t_b
        for c in chunks:
            batches.append((t0, c))
            t0 += c
    wins = [0 if j < t_lo else WW for j in range(t_b)]
    return dict(
        NPC=npc, NB=nb, N_PAD=n_pad, T_B=t_b, T_LO=t_lo, NTILES=ntiles,
        BATCHES=batches, MAXB=max(c for c in chunks), WINS=wins,
        E_PAD=ntiles * P, DVC=d_v // P,
    )


def _host_prep(nodes, edges, edge_index, Wq, Wk, Wv, cfg):
    f16 = np.float16
    NPC, N_PAD, T_B, T_LO, NTILES, E_PAD = (
        cfg["NPC"], cfg["N_PAD"], cfg["T_B"], cfg["T_LO"], cfg["NTILES"],
        cfg["E_PAD"])
    n_nodes = nodes.shape[0]
    dv = nodes.shape[1]
    DVC = cfg["DVC"]

    s = np.asarray(edge_index[0], dtype=np.int64)
    r = np.asarray(edge_index[1], dtype=np.int64)
    # sort by (receiver-block, half, receiver): edges of a 128-block come
    # low-half first, then high-half
    half_key = (r >> 6)              # global 64-group index == (blk, half)
    order = np.lexsort((r,))         # receiver-sorted
    order = order[np.argsort(half_key[order], kind="stable")]
    r_s = r[order]
    s_s = s[order]

    nodes_pad = np.zeros((N_PAD, dv), dtype=f16)
    nodes_pad[:n_nodes] = nodes.astype(f16)
    nodesT = np.ascontiguousarray(
        nodes_pad.T.reshape(DVC, P, N_PAD).transpose(1, 0, 2))
    wvT = np.ascontiguousarray(Wv.T.astype(f16))
    wkT = np.ascontiguousarray(Wk.T.reshape(DVC, P, P)).astype(f16)
    wqT = np.ascontiguousarray(Wq.T.reshape(DVC, P, P)).astype(f16)
    iota = np.tile(np.arange(WW, dtype=f16)[None, :], (P, 1))   # [128, 64]

    in_maps = []
    for c in range(N_CORES):
        lo_n, hi_n = c * NPC, (c + 1) * NPC
        sel = (r_s >= lo_n) & (r_s < hi_n)
        ids = order[sel]
        rl = r_s[sel] - lo_n
        sl = s_s[sel]
        blk = rl >> 7
        half = (rl >> 6) & 1
        # slot each (block, half) run into its subtile range
        g = blk * 2 + half                           # 64-group in [0, 2*NB)
        g_start = np.searchsorted(g, np.arange(2 * cfg["NB"]))
        within = np.arange(rl.size) - g_start[g]
        cap = np.where(np.arange(2 * cfg["NB"]) % 2 == 0, T_LO, T_B - T_LO)
        assert within.size == 0 or (within < cap[g] * P).all(), \
            "half-block overflows its subtile budget"
        base = blk * (T_B * P) + np.where(half == 0, 0, T_LO * P)
        dst = base + within

        ebuf = np.zeros((E_PAD, P), dtype=f16)
        ebuf[dst] = edges[ids].astype(f16)
        edgesT = np.ascontiguousarray(ebuf.T)        # [de, E_PAD]

        rc = np.full(E_PAD, 200.0, dtype=f16)
        rc[dst] = (rl & 63).astype(f16)              # window-local receiver
        rcolT = np.ascontiguousarray(rc.reshape(NTILES, P).T)  # [128, NTILES]

        s_arr = np.zeros(E_PAD, dtype=np.int64)
        s_arr[dst] = sl
        nodesT_e = np.ascontiguousarray(nodesT[:, :, s_arr])

        nodesT_own = np.ascontiguousarray(nodesT[:, :, lo_n:hi_n])

        in_maps.append(dict(
            edgesT=edgesT, rcolT=rcolT, nodesT_e=nodesT_e,
            nodesT_own=nodesT_own, wvT=wvT, wkT=wkT, wqT=wqT,
            iota=iota,
        ))
    return in_maps


def _pin_act_tables():
    """Restrict Bacc's activation-table choices to a single set containing
    both Exp and Lrelu, so the kernel loads the ACT table exactly once."""
    import concourse.bacc as bacc_mod
    from concourse import mybir
    if getattr(bacc_mod, "_ea_act_pinned", False):
        return
    orig = bacc_mod.get_activation_tables

    def pinned(arch):
        t = orig(arch)
        need = {mybir.ActivationFunctionType.Exp,
                mybir.ActivationFunctionType.Prelu,
                mybir.ActivationFunctionType.Relu,
                mybir.ActivationFunctionType.Copy,
                mybir.ActivationFunctionType.Identity}
        target = None
        for name, funcs in t.items():
            if need <= funcs:
                target = name
                break
        assert target is not None, "no act set with Exp+Prelu"
        return {name: (funcs if name == target else set())
                for name, funcs in t.items()}

    bacc_mod.get_activation_tables = pinned
    bacc_mod._ea_act_pinned = True


def _build_program(cfg, use_relu=False):
    import concourse.bass as bass
    import concourse.mybir as mybir
    import concourse.tile as tile
    from concourse import bacc

    _pin_act_tables()

    f16 = mybir.dt.float16
    f32 = mybir.dt.float32
    AF = mybir.ActivationFunctionType
    ACTF = AF.Relu if use_relu else AF.Prelu

    NPC, NB, N_PAD, T_B, T_LO, NTILES, E_PAD, DVC = (
        cfg["NPC"], cfg["NB"], cfg["N_PAD"], cfg["T_B"], cfg["T_LO"],
        cfg["NTILES"], cfg["E_PAD"], cfg["DVC"])
    BATCHES = cfg["BATCHES"]
    MAXB = cfg["MAXB"]
    WINS = cfg["WINS"]
    RW = P + 4                       # rhs panel stride: [v(128) | 1 | pad]
    INV_SQRT_D = 1.0 / np.sqrt(128.0)

    nc = bacc.Bacc("TRN2", target_bir_lowering=False)
    d_edgesT = nc.dram_tensor("edgesT", [P, E_PAD], f16, kind="ExternalInput")
    d_rcolT = nc.dram_tensor("rcolT", [P, NTILES], f16, kind="ExternalInput")
    d_nodesT_e = nc.dram_tensor(
        "nodesT_e", [P, DVC, E_PAD], f16, kind="ExternalInput")
    d_nodesT_own = nc.dram_tensor(
        "nodesT_own", [P, DVC, NPC], f16, kind="ExternalInput")
    d_wvT = nc.dram_tensor("wvT", [P, P], f16, kind="ExternalInput")
    d_wkT = nc.dram_tensor("wkT", [DVC, P, P], f16, kind="ExternalInput")
    d_wqT = nc.dram_tensor("wqT", [DVC, P, P], f16, kind="ExternalInput")
    d_iota = nc.dram_tensor("iota", [P, WW], f16, kind="ExternalInput")
    d_out = nc.dram_tensor("out", [NPC, P], f32, kind="ExternalOutput")

    def block_of(st):
        return min(st // T_B, NB - 1)

    with tile.TileContext(nc) as tc:
        with (
            tc.tile_pool(name="persist", bufs=1) as pp,
            tc.tile_pool(name="work", bufs=3) as wk,
            tc.tile_pool(name="rhsp", bufs=3) as rp,
            tc.tile_pool(name="edma", bufs=4) as ed,
            tc.tile_pool(name="psA", bufs=3, space="PSUM") as psA,
            tc.tile_pool(name="psO", bufs=2, space="PSUM") as psO,
        ):
            # ---- constants / persistent ----
            qT = pp.tile([P, NPC], f16, tag="qT")
            rc_all = pp.tile([P, NTILES], f16, tag="rc")
            wvT_t = pp.tile([P, P], f16, tag="wv")
            wkT_t = pp.tile([P, DVC * P], f16, tag="wkt")
            wqT_t = pp.tile([P, DVC * P], f16, tag="wqt")
            iota_t = pp.tile([P, WW], f16, tag="iota")
            nc.sync.dma_start(out=wvT_t[:], in_=d_wvT[:])
            nc.sync.dma_start(
                out=wkT_t[:].rearrange("p (c n) -> p c n", c=DVC),
                in_=d_wkT[:].rearrange("c p n -> p c n"))
            nc.sync.dma_start(
                out=wqT_t[:].rearrange("p (c n) -> p c n", c=DVC),
                in_=d_wqT[:].rearrange("c p n -> p c n"))
            nc.sync.dma_start(out=iota_t[:], in_=d_iota[:])
            nc.sync.dma_start(out=rc_all[:], in_=d_rcolT[:])

            # pre-set the ones column in every rhs-panel buffer (written
            # once; the per-batch ACT only writes cols 0..127 of each panel)
            for i in range(3):
                rb = rp.tile([P, MAXB, RW], f16, tag="rhs", name=f"rhsinit{i}")
                nc.gpsimd.memset(rb[:, :, P:P + 1], 1.0)

            # ---- phase 1: q for own nodes ----
            off = 0
            while off < NPC:
                w = min(512, NPC - off)
                qt = wk.tile([P, DVC, 512], f16, tag="qt")
                nc.sync.dma_start(
                    out=qt[:, :, :w], in_=d_nodesT_own[:, :, off:off + w])
                qps = psA.tile([P, MAXB * P], f32, tag="acc")
                for c in range(DVC):
                    nc.tensor.matmul(
                        qps[:, :w], lhsT=wqT_t[:, c * P:(c + 1) * P],
                        rhs=qt[:, c, :w], start=(c == 0), stop=(c == DVC - 1))
                nc.scalar.activation(
                    out=qT[:, off:off + w], in_=qps[:, :w],
                    func=ACTF, alpha=0.01)
                off += w

            # ---- phase 2 ----
            out_ps = {}
            for bi, (bt0, bns) in enumerate(BATCHES):
                ne = bns * P
                b = block_of(bt0)
                eT = ed.tile([P, MAXB * P], f16, tag="eT")
                nc.sync.dma_start(
                    out=eT[:, :ne], in_=d_edgesT[:, bt0 * P:bt0 * P + ne])
                ntE = ed.tile([P, DVC, MAXB * P], f16, tag="ntE")
                nc.sync.dma_start(
                    out=ntE[:, :, :ne],
                    in_=d_nodesT_e[:, :, bt0 * P:bt0 * P + ne])

                # kT_e = lrelu(Wk.T^T @ nodes_e)  [d, e]
                kps = psA.tile([P, MAXB * P], f32, tag="acc")
                for h in range(0, ne, 512):
                    hw = min(512, ne - h)
                    for c in range(DVC):
                        nc.tensor.matmul(
                            kps[:, h:h + hw],
                            lhsT=wkT_t[:, c * P:(c + 1) * P],
                            rhs=ntE[:, c, h:h + hw],
                            start=(c == 0), stop=(c == DVC - 1))
                kT = wk.tile([P, MAXB * P], f16, tag="kT")
                hne = ne // 2
                nc.scalar.activation(
                    out=kT[:, :hne], in_=kps[:, :hne], func=ACTF, alpha=0.01)
                ksc = wk.tile([P, MAXB * P // 2], f16, tag="ksc")
                nc.vector.tensor_scalar_mul(
                    out=ksc[:, :ne - hne], in0=kps[:, hne:ne], scalar1=0.01)
                nc.vector.tensor_max(
                    out=kT[:, hne:ne], in0=kps[:, hne:ne],
                    in1=ksc[:, :ne - hne])

                # v = lrelu(edges @ Wv.T) into rhs panels [v | 1]
                vps = psA.tile([P, MAXB * P], f32, tag="acc")
                for j in range(bns):
                    nc.tensor.matmul(
                        vps[:, j * P:(j + 1) * P],
                        lhsT=eT[:, j * P:(j + 1) * P],
                        rhs=wvT_t[:], start=True, stop=True)
                rhs = rp.tile([P, MAXB, RW], f16, tag="rhs")
                nc.scalar.activation(
                    out=rhs[:, :bns, :P],
                    in_=vps[:, :ne].rearrange("p (a n) -> p a n", n=P),
                    func=ACTF, alpha=0.01)

                # S = k_e . q over each subtile's 64-receiver half-block
                sps = psA.tile([P, MAXB * P], f32, tag="acc")
                for j in range(bns):
                    W = WINS[bt0 + j - b * T_B]
                    nc.tensor.matmul(
                        sps[:, j * WW:(j + 1) * WW],
                        lhsT=kT[:, j * P:(j + 1) * P],
                        rhs=qT[:, b * P + W:b * P + W + WW],
                        start=True, stop=True)
                Et = wk.tile([P, MAXB * WW], f16, tag="Et")
                nc.scalar.activation(
                    out=Et[:, :bns * WW], in_=sps[:, :bns * WW], func=AF.Exp,
                    scale=INV_SQRT_D)

                # mask: oh[e, w] = (rc_loc[e] == iota64[w]); Et *= oh
                oh = wk.tile([P, MAXB * WW], f16, tag="oh")
                nc.vector.tensor_tensor(
                    out=oh[:, :bns * WW].rearrange(
                        "p (a n) -> p a n", n=WW),
                    in0=rc_all[:, bt0:bt0 + bns, None].to_broadcast(
                        [P, bns, WW]),
                    in1=iota_t[:, None, :].to_broadcast([P, bns, WW]),
                    op=mybir.AluOpType.is_equal)
                nc.vector.tensor_mul(
                    out=Et[:, :bns * WW], in0=Et[:, :bns * WW],
                    in1=oh[:, :bns * WW])

                # out_blk[W:W+64] += P.T @ [v | 1]
                for j in range(bns):
                    st = bt0 + j
                    jb = st - b * T_B
                    W = WINS[jb]
                    if jb == 0:
                        out_ps[b] = psO.tile(
                            [P, RW], f32, tag="outp", name=f"outp{b}")
                    first = jb == 0 or jb == T_LO
                    last = jb == T_LO - 1 or jb == T_B - 1
                    nc.tensor.matmul(
                        out_ps[b][W:W + WW, :P + 1],
                        lhsT=Et[:, j * WW:(j + 1) * WW],
                        rhs=rhs[:, j, :P + 1],
                        start=first, stop=last,
                        skip_group_check=True)
                    if jb == T_B - 1:
                        rec = wk.tile([P, 1], f32, tag="rec")
                        nc.vector.reciprocal(rec[:], out_ps[b][:, P:P + 1])
                        o = wk.tile([P, P], f32, tag="o")
                        nc.vector.tensor_scalar_mul(
                            out=o[:], in0=out_ps[b][:, :P], scalar1=rec[:])
                        nc.sync.dma_start(
                            out=d_out[b * P:(b + 1) * P, :], in_=o[:])
                        del out_ps[b]

    nc.compile()
    return nc


def _budgets(edge_index, n_nodes):
    """Per-64-group subtile budgets (t_lo, t_hi), maxed over cores/blocks."""
    r = np.asarray(edge_index[1], dtype=np.int64)
    npc = -(-n_nodes // (N_CORES * P)) * P
    while (npc * N_CORES) % 512:
        npc += P
    cnt64 = np.bincount(r >> 6, minlength=(npc * N_CORES) >> 6)
    lo = cnt64[0::2]
    hi = cnt64[1::2]
    t_lo = max(1, int(-(-lo.max() // P)))
    t_hi = max(1, int(-(-hi.max() // P)))
    return t_lo, t_hi


def kernel(nodes, edges, edge_index, Wq, bq, Wk, bk, Wv, bv, **_unused):
    nodes = np.asarray(nodes)
    edges = np.asarray(edges)
    edge_index = np.asarray(edge_index)
    n_nodes, d_v = nodes.shape
    n_edges, d_e = edges.shape
    d_attn = Wq.shape[0]
    assert not np.any(bq) and not np.any(bk) and not np.any(bv), \
        "zero biases assumed"

    t_lo, t_hi = _budgets(edge_index, n_nodes)
    cfg = _cfg_from_shapes(n_nodes, n_edges, d_v, d_e, d_attn, t_lo, t_hi)

    in_maps = _host_prep(nodes, edges, edge_index,
                         np.asarray(Wq), np.asarray(Wk), np.asarray(Wv), cfg)
    nc = _build_program(cfg)

    from concourse.bass_utils import run_bass_kernel_spmd
    res = run_bass_kernel_spmd(nc, in_maps, core_ids=list(range(N_CORES)))
    out = np.concatenate([res.results[c]["out"] for c in range(N_CORES)], axis=0)
    return np.ascontiguousarray(out[:n_nodes]).astype(np.float32)


# revision 23
# speedup vs baseline: 1.3147x; 1.3147x over previous
"""EdgeAttention GNN message passing on 8 Trainium2 NeuronCores.

Strategy (edge-parallel, receiver-sorted, gather-free, half-block windows):
  - Host: sort edges by receiver node, shard NODES uniformly across the 8
    cores (each core owns a contiguous 1/8 node range); each core processes
    exactly the edges whose receiver it owns (~E/8 by symmetry). Within a
    core, receivers are grouped into blocks of 128 nodes; each block's edges
    are split into a low half (receivers 0..63) and high half (64..127),
    each padded to a fixed number of 128-edge subtiles (uniform across
    cores/blocks so the SPMD program is identical). The host materializes
    the SENDER's raw node features per edge slot (pure data movement), so
    the device never needs an indirect gather (the Q7 descriptor path was
    measured at ~10 ns/edge serialized).
  - Device phase 1: q = lrelu(own_nodes @ Wq.T) kept in SBUF.
  - Device phase 2, per <=8-subtile batch:
      kT_e     = lrelu(Wk.T^T @ nodesT_e)     (PE w/ stationary weights,
                                               512-col rhs + one ACT)
      v        = lrelu(edgesT @ Wv.T)         (PE per subtile + one ACT)
      S        = kT_e.T @ q_half              (PE per subtile, 64 cols)
      Et       = exp(S / sqrt(d))             (one ACT per batch, 64*bns)
      oh       = is_equal(rc_loc, iota64)     (one DVE TT per batch)
      P        = Et * oh                      (DVE tensor_tensor)
      out_blk[W:W+64] += P.T @ [v | 1]        (PE; col 128 = softmax denom)
    Block epilogue: out = numer * reciprocal(denom), DMA to DRAM.
  Softmax max-subtraction is skipped: logits are O(1) here and
  exp(l)/sum(exp(l)) == exp(l-m)/sum(exp(l-m)) exactly in the reals.
"""

import sys

sys.path.insert(0, "/opt/trn_rl_repo")

import numpy as np

N_CORES = 8
P = 128
WW = 32                              # receiver window width per subtile


def _cfg_from_shapes(n_nodes, n_edges, d_v, d_e, d_attn, t_list):
    assert d_v % P == 0 and d_e == P and d_attn == P
    npc = -(-n_nodes // (N_CORES * P)) * P          # nodes per core, mult of 128
    while (npc * N_CORES) % 512:
        npc += P
    nb = npc // P                                   # blocks per core
    n_pad = npc * N_CORES
    t_b = sum(t_list)
    ntiles = nb * t_b
    # batches: per-block chunks of <=8 subtiles (never cross a block)
    chunks = []
    left = t_b
    while left > 0:
        c = min(8, left)
        if left - c == 1:
            c -= 1
        chunks.append(c)
        left -= c
    batches = []                     # (start_subtile, n_subtiles)
    for b in range(nb):
        t0 = b * t_b
        for c in chunks:
            batches.append((t0, c))
            t0 += c
    wins = []
    starts = []
    t0 = 0
    for g, t in enumerate(t_list):
        starts.append(t0)
        wins += [g * WW] * t
        t0 += t
    return dict(
        NPC=npc, NB=nb, N_PAD=n_pad, T_B=t_b, T_LIST=list(t_list),
        T_STARTS=starts, NTILES=ntiles,
        BATCHES=batches, MAXB=max(c for c in chunks), WINS=wins,
        E_PAD=ntiles * P, DVC=d_v // P,
    )


def _host_prep(nodes, edges, edge_index, Wq, Wk, Wv, cfg):
    f16 = np.float16
    NPC, N_PAD, T_B, NTILES, E_PAD = (
        cfg["NPC"], cfg["N_PAD"], cfg["T_B"], cfg["NTILES"], cfg["E_PAD"])
    T_LIST = cfg["T_LIST"]
    T_STARTS = cfg["T_STARTS"]
    G = P // WW
    n_nodes = nodes.shape[0]
    dv = nodes.shape[1]
    DVC = cfg["DVC"]

    s = np.asarray(edge_index[0], dtype=np.int64)
    r = np.asarray(edge_index[1], dtype=np.int64)
    # receiver-sorted (which also orders the within-block window groups)
    order = np.argsort(r, kind="stable")
    r_s = r[order]
    s_s = s[order]

    nodes_pad = np.zeros((N_PAD, dv), dtype=f16)
    nodes_pad[:n_nodes] = nodes.astype(f16)
    nodesT = np.ascontiguousarray(
        nodes_pad.T.reshape(DVC, P, N_PAD).transpose(1, 0, 2))
    wvT = np.ascontiguousarray(Wv.T.astype(f16))
    wkT = np.ascontiguousarray(Wk.T.reshape(DVC, P, P)).astype(f16)
    wqT = np.ascontiguousarray(Wq.T.reshape(DVC, P, P)).astype(f16)
    iota = np.tile(np.arange(WW, dtype=f16)[None, :], (P, 1))   # [128, 64]

    in_maps = []
    for c in range(N_CORES):
        lo_n, hi_n = c * NPC, (c + 1) * NPC
        sel = (r_s >= lo_n) & (r_s < hi_n)
        ids = order[sel]
        rl = r_s[sel] - lo_n
        sl = s_s[sel]
        blk = rl >> 7
        grp = (rl & 127) // WW
        # slot each (block, window-group) run into its subtile range
        g = blk * G + grp                            # window-group index
        g_start = np.searchsorted(g, np.arange(G * cfg["NB"]))
        within = np.arange(rl.size) - g_start[g]
        cap = np.asarray(T_LIST, dtype=np.int64)[np.arange(G * cfg["NB"]) % G]
        assert within.size == 0 or (within < cap[g] * P).all(), \
            "window group overflows its subtile budget"
        base = blk * (T_B * P) + np.asarray(T_STARTS, dtype=np.int64)[grp] * P
        dst = base + within

        ebuf = np.zeros((E_PAD, P), dtype=f16)
        ebuf[dst] = edges[ids].astype(f16)
        edgesT = np.ascontiguousarray(ebuf.T)        # [de, E_PAD]

        rc = np.full(E_PAD, 200.0, dtype=f16)
        rc[dst] = (rl % WW).astype(f16)              # window-local receiver
        rcolT = np.ascontiguousarray(rc.reshape(NTILES, P).T)  # [128, NTILES]

        s_arr = np.zeros(E_PAD, dtype=np.int64)
        s_arr[dst] = sl
        nodesT_e = np.ascontiguousarray(nodesT[:, :, s_arr])

        nodesT_own = np.ascontiguousarray(nodesT[:, :, lo_n:hi_n])

        in_maps.append(dict(
            edgesT=edgesT, rcolT=rcolT, nodesT_e=nodesT_e,
            nodesT_own=nodesT_own, wvT=wvT, wkT=wkT, wqT=wqT,
            iota=iota,
        ))
    return in_maps


def _pin_act_tables():
    """Restrict Bacc's activation-table choices to a single set containing
    both Exp and Lrelu, so the kernel loads the ACT table exactly once."""
    import concourse.bacc as bacc_mod
    from concourse import mybir
    if getattr(bacc_mod, "_ea_act_pinned", False):
        return
    orig = bacc_mod.get_activation_tables

    def pinned(arch):
        t = orig(arch)
        need = {mybir.ActivationFunctionType.Exp,
                mybir.ActivationFunctionType.Prelu,
                mybir.ActivationFunctionType.Relu,
                mybir.ActivationFunctionType.Copy,
                mybir.ActivationFunctionType.Identity}
        target = None
        for name, funcs in t.items():
            if need <= funcs:
                target = name
                break
        assert target is not None, "no act set with Exp+Prelu"
        return {name: (funcs if name == target else set())
                for name, funcs in t.items()}

    bacc_mod.get_activation_tables = pinned
    bacc_mod._ea_act_pinned = True


def _build_program(cfg, use_relu=False):
    import concourse.bass as bass
    import concourse.mybir as mybir
    import concourse.tile as tile
    from concourse import bacc

    _pin_act_tables()

    f16 = mybir.dt.float16
    f32 = mybir.dt.float32
    AF = mybir.ActivationFunctionType
    ACTF = AF.Relu if use_relu else AF.Prelu

    NPC, NB, N_PAD, T_B, NTILES, E_PAD, DVC = (
        cfg["NPC"], cfg["NB"], cfg["N_PAD"], cfg["T_B"],
        cfg["NTILES"], cfg["E_PAD"], cfg["DVC"])
    T_STARTS = cfg["T_STARTS"]
    T_LIST = cfg["T_LIST"]
    GRP_OF = []
    for g, t in enumerate(T_LIST):
        GRP_OF += [g] * t
    BATCHES = cfg["BATCHES"]
    MAXB = cfg["MAXB"]
    WINS = cfg["WINS"]
    RW = P + 4                       # rhs panel stride: [v(128) | 1 | pad]
    INV_SQRT_D = 1.0 / np.sqrt(128.0)

    nc = bacc.Bacc("TRN2", target_bir_lowering=False)
    d_edgesT = nc.dram_tensor("edgesT", [P, E_PAD], f16, kind="ExternalInput")
    d_rcolT = nc.dram_tensor("rcolT", [P, NTILES], f16, kind="ExternalInput")
    d_nodesT_e = nc.dram_tensor(
        "nodesT_e", [P, DVC, E_PAD], f16, kind="ExternalInput")
    d_nodesT_own = nc.dram_tensor(
        "nodesT_own", [P, DVC, NPC], f16, kind="ExternalInput")
    d_wvT = nc.dram_tensor("wvT", [P, P], f16, kind="ExternalInput")
    d_wkT = nc.dram_tensor("wkT", [DVC, P, P], f16, kind="ExternalInput")
    d_wqT = nc.dram_tensor("wqT", [DVC, P, P], f16, kind="ExternalInput")
    d_iota = nc.dram_tensor("iota", [P, WW], f16, kind="ExternalInput")
    d_out = nc.dram_tensor("out", [NPC, P], f32, kind="ExternalOutput")

    def block_of(st):
        return min(st // T_B, NB - 1)

    with tile.TileContext(nc) as tc:
        with (
            tc.tile_pool(name="persist", bufs=1) as pp,
            tc.tile_pool(name="work", bufs=3) as wk,
            tc.tile_pool(name="rhsp", bufs=3) as rp,
            tc.tile_pool(name="edma", bufs=4) as ed,
            tc.tile_pool(name="psA", bufs=3, space="PSUM") as psA,
            tc.tile_pool(name="psO", bufs=2, space="PSUM") as psO,
        ):
            # ---- constants / persistent ----
            qT = pp.tile([P, NPC], f16, tag="qT")
            rc_all = pp.tile([P, NTILES], f16, tag="rc")
            wvT_t = pp.tile([P, P], f16, tag="wv")
            wkT_t = pp.tile([P, DVC * P], f16, tag="wkt")
            wqT_t = pp.tile([P, DVC * P], f16, tag="wqt")
            iota_t = pp.tile([P, WW], f16, tag="iota")
            nc.sync.dma_start(out=wvT_t[:], in_=d_wvT[:])
            nc.sync.dma_start(
                out=wkT_t[:].rearrange("p (c n) -> p c n", c=DVC),
                in_=d_wkT[:].rearrange("c p n -> p c n"))
            nc.sync.dma_start(
                out=wqT_t[:].rearrange("p (c n) -> p c n", c=DVC),
                in_=d_wqT[:].rearrange("c p n -> p c n"))
            nc.sync.dma_start(out=iota_t[:], in_=d_iota[:])
            nc.sync.dma_start(out=rc_all[:], in_=d_rcolT[:])

            # pre-set the ones column in every rhs-panel buffer (written
            # once; the per-batch ACT only writes cols 0..127 of each panel)
            for i in range(3):
                rb = rp.tile([P, MAXB, RW], f16, tag="rhs", name=f"rhsinit{i}")
                nc.gpsimd.memset(rb[:, :, P:P + 1], 1.0)

            # ---- phase 1: q for own nodes ----
            off = 0
            while off < NPC:
                w = min(512, NPC - off)
                qt = wk.tile([P, DVC, 512], f16, tag="qt")
                nc.sync.dma_start(
                    out=qt[:, :, :w], in_=d_nodesT_own[:, :, off:off + w])
                qps = psA.tile([P, MAXB * P], f32, tag="acc")
                for c in range(DVC):
                    nc.tensor.matmul(
                        qps[:, :w], lhsT=wqT_t[:, c * P:(c + 1) * P],
                        rhs=qt[:, c, :w], start=(c == 0), stop=(c == DVC - 1))
                nc.scalar.activation(
                    out=qT[:, off:off + w], in_=qps[:, :w],
                    func=ACTF, alpha=0.01)
                off += w

            # ---- phase 2 ----
            out_ps = {}
            for bi, (bt0, bns) in enumerate(BATCHES):
                ne = bns * P
                b = block_of(bt0)
                eT = ed.tile([P, MAXB * P], f16, tag="eT")
                nc.sync.dma_start(
                    out=eT[:, :ne], in_=d_edgesT[:, bt0 * P:bt0 * P + ne])
                ntE = ed.tile([P, DVC, MAXB * P], f16, tag="ntE")
                nc.sync.dma_start(
                    out=ntE[:, :, :ne],
                    in_=d_nodesT_e[:, :, bt0 * P:bt0 * P + ne])

                # kT_e = lrelu(Wk.T^T @ nodes_e)  [d, e]
                kps = psA.tile([P, MAXB * P], f32, tag="acc")
                for h in range(0, ne, 512):
                    hw = min(512, ne - h)
                    for c in range(DVC):
                        nc.tensor.matmul(
                            kps[:, h:h + hw],
                            lhsT=wkT_t[:, c * P:(c + 1) * P],
                            rhs=ntE[:, c, h:h + hw],
                            start=(c == 0), stop=(c == DVC - 1))
                kT = wk.tile([P, MAXB * P], f16, tag="kT")
                nc.scalar.activation(
                    out=kT[:, :ne], in_=kps[:, :ne], func=ACTF, alpha=0.01)

                # v = lrelu(edges @ Wv.T) into rhs panels [v | 1]
                vps = psA.tile([P, MAXB * P], f32, tag="acc")
                for j in range(bns):
                    nc.tensor.matmul(
                        vps[:, j * P:(j + 1) * P],
                        lhsT=eT[:, j * P:(j + 1) * P],
                        rhs=wvT_t[:], start=True, stop=True)
                rhs = rp.tile([P, MAXB, RW], f16, tag="rhs")
                nc.scalar.activation(
                    out=rhs[:, :bns, :P],
                    in_=vps[:, :ne].rearrange("p (a n) -> p a n", n=P),
                    func=ACTF, alpha=0.01)

                # S = k_e . q over each subtile's 64-receiver half-block
                sps = psA.tile([P, MAXB * P], f32, tag="acc")
                for j in range(bns):
                    W = WINS[bt0 + j - b * T_B]
                    nc.tensor.matmul(
                        sps[:, j * WW:(j + 1) * WW],
                        lhsT=kT[:, j * P:(j + 1) * P],
                        rhs=qT[:, b * P + W:b * P + W + WW],
                        start=True, stop=True)
                Et = wk.tile([P, MAXB * WW], f16, tag="Et")
                nc.scalar.activation(
                    out=Et[:, :bns * WW], in_=sps[:, :bns * WW], func=AF.Exp,
                    scale=INV_SQRT_D)

                # mask: oh[e, w] = (rc_loc[e] == iota64[w]); Et *= oh
                oh = wk.tile([P, MAXB * WW], f16, tag="oh")
                nc.vector.tensor_tensor(
                    out=oh[:, :bns * WW].rearrange(
                        "p (a n) -> p a n", n=WW),
                    in0=rc_all[:, bt0:bt0 + bns, None].to_broadcast(
                        [P, bns, WW]),
                    in1=iota_t[:, None, :].to_broadcast([P, bns, WW]),
                    op=mybir.AluOpType.is_equal)
                nc.vector.tensor_mul(
                    out=Et[:, :bns * WW], in0=Et[:, :bns * WW],
                    in1=oh[:, :bns * WW])

                # out_blk[W:W+64] += P.T @ [v | 1]
                for j in range(bns):
                    st = bt0 + j
                    jb = st - b * T_B
                    W = WINS[jb]
                    if jb == 0:
                        out_ps[b] = psO.tile(
                            [P, RW], f32, tag="outp", name=f"outp{b}")
                    gg = GRP_OF[jb]
                    first = jb == T_STARTS[gg]
                    last = jb == T_STARTS[gg] + T_LIST[gg] - 1
                    nc.tensor.matmul(
                        out_ps[b][W:W + WW, :P + 1],
                        lhsT=Et[:, j * WW:(j + 1) * WW],
                        rhs=rhs[:, j, :P + 1],
                        start=first, stop=last,
                        tile_position=(0, W),
                        skip_group_check=True)
                    if jb == T_B - 1:
                        rec = wk.tile([P, 1], f32, tag="rec")
                        nc.vector.reciprocal(rec[:], out_ps[b][:, P:P + 1])
                        o = wk.tile([P, P], f32, tag="o")
                        nc.vector.tensor_scalar_mul(
                            out=o[:], in0=out_ps[b][:, :P], scalar1=rec[:])
                        nc.sync.dma_start(
                            out=d_out[b * P:(b + 1) * P, :], in_=o[:])
                        del out_ps[b]

    nc.compile()
    return nc


def _budgets(edge_index, n_nodes):
    """Per-window-group subtile budgets, maxed over cores/blocks."""
    G = P // WW
    r = np.asarray(edge_index[1], dtype=np.int64)
    npc = -(-n_nodes // (N_CORES * P)) * P
    while (npc * N_CORES) % 512:
        npc += P
    cnt = np.bincount(r // WW, minlength=(npc * N_CORES) // WW)
    return [max(1, int(-(-cnt[g::G].max() // P))) for g in range(G)]


def kernel(nodes, edges, edge_index, Wq, bq, Wk, bk, Wv, bv, **_unused):
    nodes = np.asarray(nodes)
    edges = np.asarray(edges)
    edge_index = np.asarray(edge_index)
    n_nodes, d_v = nodes.shape
    n_edges, d_e = edges.shape
    d_attn = Wq.shape[0]
    assert not np.any(bq) and not np.any(bk) and not np.any(bv), \
        "zero biases assumed"

    t_list = _budgets(edge_index, n_nodes)
    cfg = _cfg_from_shapes(n_nodes, n_edges, d_v, d_e, d_attn, t_list)

    in_maps = _host_prep(nodes, edges, edge_index,
                         np.asarray(Wq), np.asarray(Wk), np.asarray(Wv), cfg)
    nc = _build_program(cfg)

    from concourse.bass_utils import run_bass_kernel_spmd
    res = run_bass_kernel_spmd(nc, in_maps, core_ids=list(range(N_CORES)))
    out = np.concatenate([res.results[c]["out"] for c in range(N_CORES)], axis=0)
    return np.ascontiguousarray(out[:n_nodes]).astype(np.float32)


# revision 24
# speedup vs baseline: 1.3173x; 1.0020x over previous
"""EdgeAttention GNN message passing on 8 Trainium2 NeuronCores.

Strategy (edge-parallel, receiver-sorted, gather-free, half-block windows):
  - Host: sort edges by receiver node, shard NODES uniformly across the 8
    cores (each core owns a contiguous 1/8 node range); each core processes
    exactly the edges whose receiver it owns (~E/8 by symmetry). Within a
    core, receivers are grouped into blocks of 128 nodes; each block's edges
    are split into a low half (receivers 0..63) and high half (64..127),
    each padded to a fixed number of 128-edge subtiles (uniform across
    cores/blocks so the SPMD program is identical). The host materializes
    the SENDER's raw node features per edge slot (pure data movement), so
    the device never needs an indirect gather (the Q7 descriptor path was
    measured at ~10 ns/edge serialized).
  - Device phase 1: q = lrelu(own_nodes @ Wq.T) kept in SBUF.
  - Device phase 2, per <=8-subtile batch:
      kT_e     = lrelu(Wk.T^T @ nodesT_e)     (PE w/ stationary weights,
                                               512-col rhs + one ACT)
      v        = lrelu(edgesT @ Wv.T)         (PE per subtile + one ACT)
      S        = kT_e.T @ q_half              (PE per subtile, 64 cols)
      Et       = exp(S / sqrt(d))             (one ACT per batch, 64*bns)
      oh       = is_equal(rc_loc, iota64)     (one DVE TT per batch)
      P        = Et * oh                      (DVE tensor_tensor)
      out_blk[W:W+64] += P.T @ [v | 1]        (PE; col 128 = softmax denom)
    Block epilogue: out = numer * reciprocal(denom), DMA to DRAM.
  Softmax max-subtraction is skipped: logits are O(1) here and
  exp(l)/sum(exp(l)) == exp(l-m)/sum(exp(l-m)) exactly in the reals.
"""

import sys

sys.path.insert(0, "/opt/trn_rl_repo")

import numpy as np

N_CORES = 8
P = 128
WW = 32                              # receiver window width per subtile


def _cfg_from_shapes(n_nodes, n_edges, d_v, d_e, d_attn, t_list):
    assert d_v % P == 0 and d_e == P and d_attn == P
    npc = -(-n_nodes // (N_CORES * P)) * P          # nodes per core, mult of 128
    while (npc * N_CORES) % 512:
        npc += P
    nb = npc // P                                   # blocks per core
    n_pad = npc * N_CORES
    t_b = sum(t_list)
    ntiles = nb * t_b
    # batches: per-block chunks of <=8 subtiles (never cross a block)
    chunks = []
    left = t_b
    while left > 0:
        c = min(8, left)
        if left - c == 1:
            c -= 1
        chunks.append(c)
        left -= c
    batches = []                     # (start_subtile, n_subtiles)
    for b in range(nb):
        t0 = b * t_b
        for c in chunks:
            batches.append((t0, c))
            t0 += c
    wins = []
    starts = []
    t0 = 0
    for g, t in enumerate(t_list):
        starts.append(t0)
        wins += [g * WW] * t
        t0 += t
    return dict(
        NPC=npc, NB=nb, N_PAD=n_pad, T_B=t_b, T_LIST=list(t_list),
        T_STARTS=starts, NTILES=ntiles,
        BATCHES=batches, MAXB=max(c for c in chunks), WINS=wins,
        E_PAD=ntiles * P, DVC=d_v // P,
    )


def _host_prep(nodes, edges, edge_index, Wq, Wk, Wv, cfg):
    import ml_dtypes
    f8 = ml_dtypes.float8_e4m3
    f16 = np.float16
    NPC, N_PAD, T_B, NTILES, E_PAD = (
        cfg["NPC"], cfg["N_PAD"], cfg["T_B"], cfg["NTILES"], cfg["E_PAD"])
    T_LIST = cfg["T_LIST"]
    T_STARTS = cfg["T_STARTS"]
    G = P // WW
    n_nodes = nodes.shape[0]
    dv = nodes.shape[1]
    DVC = cfg["DVC"]

    s = np.asarray(edge_index[0], dtype=np.int64)
    r = np.asarray(edge_index[1], dtype=np.int64)
    # receiver-sorted (which also orders the within-block window groups)
    order = np.argsort(r, kind="stable")
    r_s = r[order]
    s_s = s[order]

    nodes_pad = np.zeros((N_PAD, dv), dtype=f16)
    nodes_pad[:n_nodes] = nodes.astype(f16)
    nodesT = np.ascontiguousarray(
        nodes_pad.T.reshape(DVC, P, N_PAD).transpose(1, 0, 2))
    nodesT8 = nodesT.astype(f8)
    wvT = np.ascontiguousarray(Wv.T.astype(f16))
    wkT = np.ascontiguousarray(Wk.T.reshape(DVC, P, P)).astype(f8)
    wqT = np.ascontiguousarray(Wq.T.reshape(DVC, P, P)).astype(f16)
    iota = np.tile(np.arange(WW, dtype=f16)[None, :], (P, 1))   # [128, 64]

    in_maps = []
    for c in range(N_CORES):
        lo_n, hi_n = c * NPC, (c + 1) * NPC
        sel = (r_s >= lo_n) & (r_s < hi_n)
        ids = order[sel]
        rl = r_s[sel] - lo_n
        sl = s_s[sel]
        blk = rl >> 7
        grp = (rl & 127) // WW
        # slot each (block, window-group) run into its subtile range
        g = blk * G + grp                            # window-group index
        g_start = np.searchsorted(g, np.arange(G * cfg["NB"]))
        within = np.arange(rl.size) - g_start[g]
        cap = np.asarray(T_LIST, dtype=np.int64)[np.arange(G * cfg["NB"]) % G]
        assert within.size == 0 or (within < cap[g] * P).all(), \
            "window group overflows its subtile budget"
        base = blk * (T_B * P) + np.asarray(T_STARTS, dtype=np.int64)[grp] * P
        dst = base + within

        ebuf = np.zeros((E_PAD, P), dtype=f16)
        ebuf[dst] = edges[ids].astype(f16)
        edgesT = np.ascontiguousarray(ebuf.T)        # [de, E_PAD]

        rc = np.full(E_PAD, 200.0, dtype=f16)
        rc[dst] = (rl % WW).astype(f16)              # window-local receiver
        rcolT = np.ascontiguousarray(rc.reshape(NTILES, P).T)  # [128, NTILES]

        s_arr = np.zeros(E_PAD, dtype=np.int64)
        s_arr[dst] = sl
        nodesT_e = np.ascontiguousarray(nodesT8[:, :, s_arr])

        nodesT_own = np.ascontiguousarray(nodesT[:, :, lo_n:hi_n])

        in_maps.append(dict(
            edgesT=edgesT, rcolT=rcolT, nodesT_e=nodesT_e,
            nodesT_own=nodesT_own, wvT=wvT, wkT=wkT, wqT=wqT,
            iota=iota,
        ))
    return in_maps


def _pin_act_tables():
    """Restrict Bacc's activation-table choices to a single set containing
    both Exp and Lrelu, so the kernel loads the ACT table exactly once."""
    import concourse.bacc as bacc_mod
    from concourse import mybir
    if getattr(bacc_mod, "_ea_act_pinned", False):
        return
    orig = bacc_mod.get_activation_tables

    def pinned(arch):
        t = orig(arch)
        need = {mybir.ActivationFunctionType.Exp,
                mybir.ActivationFunctionType.Prelu,
                mybir.ActivationFunctionType.Relu,
                mybir.ActivationFunctionType.Copy,
                mybir.ActivationFunctionType.Identity}
        target = None
        for name, funcs in t.items():
            if need <= funcs:
                target = name
                break
        assert target is not None, "no act set with Exp+Prelu"
        return {name: (funcs if name == target else set())
                for name, funcs in t.items()}

    bacc_mod.get_activation_tables = pinned
    bacc_mod._ea_act_pinned = True


def _build_program(cfg, use_relu=False):
    import concourse.bass as bass
    import concourse.mybir as mybir
    import concourse.tile as tile
    from concourse import bacc

    _pin_act_tables()

    f16 = mybir.dt.float16
    f32 = mybir.dt.float32
    f8 = mybir.dt.float8e4
    AF = mybir.ActivationFunctionType
    ACTF = AF.Relu if use_relu else AF.Prelu

    NPC, NB, N_PAD, T_B, NTILES, E_PAD, DVC = (
        cfg["NPC"], cfg["NB"], cfg["N_PAD"], cfg["T_B"],
        cfg["NTILES"], cfg["E_PAD"], cfg["DVC"])
    T_STARTS = cfg["T_STARTS"]
    T_LIST = cfg["T_LIST"]
    GRP_OF = []
    for g, t in enumerate(T_LIST):
        GRP_OF += [g] * t
    BATCHES = cfg["BATCHES"]
    MAXB = cfg["MAXB"]
    WINS = cfg["WINS"]
    RW = P + 4                       # rhs panel stride: [v(128) | 1 | pad]
    INV_SQRT_D = 1.0 / np.sqrt(128.0)

    nc = bacc.Bacc("TRN2", target_bir_lowering=False)
    d_edgesT = nc.dram_tensor("edgesT", [P, E_PAD], f16, kind="ExternalInput")
    d_rcolT = nc.dram_tensor("rcolT", [P, NTILES], f16, kind="ExternalInput")
    d_nodesT_e = nc.dram_tensor(
        "nodesT_e", [P, DVC, E_PAD], f8, kind="ExternalInput")
    d_nodesT_own = nc.dram_tensor(
        "nodesT_own", [P, DVC, NPC], f16, kind="ExternalInput")
    d_wvT = nc.dram_tensor("wvT", [P, P], f16, kind="ExternalInput")
    d_wkT = nc.dram_tensor("wkT", [DVC, P, P], f8, kind="ExternalInput")
    d_wqT = nc.dram_tensor("wqT", [DVC, P, P], f16, kind="ExternalInput")
    d_iota = nc.dram_tensor("iota", [P, WW], f16, kind="ExternalInput")
    d_out = nc.dram_tensor("out", [NPC, P], f32, kind="ExternalOutput")

    def block_of(st):
        return min(st // T_B, NB - 1)

    with tile.TileContext(nc) as tc:
        with (
            tc.tile_pool(name="persist", bufs=1) as pp,
            tc.tile_pool(name="work", bufs=3) as wk,
            tc.tile_pool(name="rhsp", bufs=3) as rp,
            tc.tile_pool(name="edma", bufs=4) as ed,
            tc.tile_pool(name="psA", bufs=3, space="PSUM") as psA,
            tc.tile_pool(name="psO", bufs=2, space="PSUM") as psO,
        ):
            # ---- constants / persistent ----
            qT = pp.tile([P, NPC], f16, tag="qT")
            rc_all = pp.tile([P, NTILES], f16, tag="rc")
            wvT_t = pp.tile([P, P], f16, tag="wv")
            wkT_t = pp.tile([P, DVC * P], f8, tag="wkt")
            wqT_t = pp.tile([P, DVC * P], f16, tag="wqt")
            iota_t = pp.tile([P, WW], f16, tag="iota")
            nc.sync.dma_start(out=wvT_t[:], in_=d_wvT[:])
            nc.sync.dma_start(
                out=wkT_t[:].rearrange("p (c n) -> p c n", c=DVC),
                in_=d_wkT[:].rearrange("c p n -> p c n"))
            nc.sync.dma_start(
                out=wqT_t[:].rearrange("p (c n) -> p c n", c=DVC),
                in_=d_wqT[:].rearrange("c p n -> p c n"))
            nc.sync.dma_start(out=iota_t[:], in_=d_iota[:])
            nc.sync.dma_start(out=rc_all[:], in_=d_rcolT[:])

            # pre-set the ones column in every rhs-panel buffer (written
            # once; the per-batch ACT only writes cols 0..127 of each panel)
            for i in range(3):
                rb = rp.tile([P, MAXB, RW], f16, tag="rhs", name=f"rhsinit{i}")
                nc.gpsimd.memset(rb[:, :, P:P + 1], 1.0)

            # ---- phase 1: q for own nodes ----
            off = 0
            while off < NPC:
                w = min(512, NPC - off)
                qt = wk.tile([P, DVC, 512], f16, tag="qt")
                nc.sync.dma_start(
                    out=qt[:, :, :w], in_=d_nodesT_own[:, :, off:off + w])
                qps = psA.tile([P, MAXB * P], f32, tag="acc")
                for c in range(DVC):
                    nc.tensor.matmul(
                        qps[:, :w], lhsT=wqT_t[:, c * P:(c + 1) * P],
                        rhs=qt[:, c, :w], start=(c == 0), stop=(c == DVC - 1))
                nc.scalar.activation(
                    out=qT[:, off:off + w], in_=qps[:, :w],
                    func=ACTF, alpha=0.01)
                off += w

            # ---- phase 2 ----
            out_ps = {}
            for bi, (bt0, bns) in enumerate(BATCHES):
                ne = bns * P
                b = block_of(bt0)
                eT = ed.tile([P, MAXB * P], f16, tag="eT")
                nc.sync.dma_start(
                    out=eT[:, :ne], in_=d_edgesT[:, bt0 * P:bt0 * P + ne])
                ntE = ed.tile([P, DVC, MAXB * P], f8, tag="ntE")
                nc.sync.dma_start(
                    out=ntE[:, :, :ne],
                    in_=d_nodesT_e[:, :, bt0 * P:bt0 * P + ne])

                # kT_e = lrelu(Wk.T^T @ nodes_e)  [d, e]
                kps = psA.tile([P, MAXB * P], f32, tag="acc")
                for h in range(0, ne, 512):
                    hw = min(512, ne - h)
                    for c in range(DVC):
                        nc.tensor.matmul(
                            kps[:, h:h + hw],
                            lhsT=wkT_t[:, c * P:(c + 1) * P],
                            rhs=ntE[:, c, h:h + hw],
                            start=(c == 0), stop=(c == DVC - 1))
                kT = wk.tile([P, MAXB * P], f16, tag="kT")
                nc.scalar.activation(
                    out=kT[:, :ne], in_=kps[:, :ne], func=ACTF, alpha=0.01)

                # v = lrelu(edges @ Wv.T) into rhs panels [v | 1]
                vps = psA.tile([P, MAXB * P], f32, tag="acc")
                for j in range(bns):
                    nc.tensor.matmul(
                        vps[:, j * P:(j + 1) * P],
                        lhsT=eT[:, j * P:(j + 1) * P],
                        rhs=wvT_t[:], start=True, stop=True)
                rhs = rp.tile([P, MAXB, RW], f16, tag="rhs")
                nc.scalar.activation(
                    out=rhs[:, :bns, :P],
                    in_=vps[:, :ne].rearrange("p (a n) -> p a n", n=P),
                    func=ACTF, alpha=0.01)

                # S = k_e . q over each subtile's 64-receiver half-block
                sps = psA.tile([P, MAXB * P], f32, tag="acc")
                for j in range(bns):
                    W = WINS[bt0 + j - b * T_B]
                    nc.tensor.matmul(
                        sps[:, j * WW:(j + 1) * WW],
                        lhsT=kT[:, j * P:(j + 1) * P],
                        rhs=qT[:, b * P + W:b * P + W + WW],
                        start=True, stop=True)
                Et = wk.tile([P, MAXB * WW], f16, tag="Et")
                nc.scalar.activation(
                    out=Et[:, :bns * WW], in_=sps[:, :bns * WW], func=AF.Exp,
                    scale=INV_SQRT_D)

                # mask: oh[e, w] = (rc_loc[e] == iota64[w]); Et *= oh
                oh = wk.tile([P, MAXB * WW], f16, tag="oh")
                nc.vector.tensor_tensor(
                    out=oh[:, :bns * WW].rearrange(
                        "p (a n) -> p a n", n=WW),
                    in0=rc_all[:, bt0:bt0 + bns, None].to_broadcast(
                        [P, bns, WW]),
                    in1=iota_t[:, None, :].to_broadcast([P, bns, WW]),
                    op=mybir.AluOpType.is_equal)
                nc.vector.tensor_mul(
                    out=Et[:, :bns * WW], in0=Et[:, :bns * WW],
                    in1=oh[:, :bns * WW])

                # out_blk[W:W+64] += P.T @ [v | 1]
                for j in range(bns):
                    st = bt0 + j
                    jb = st - b * T_B
                    W = WINS[jb]
                    if jb == 0:
                        out_ps[b] = psO.tile(
                            [P, RW], f32, tag="outp", name=f"outp{b}")
                    gg = GRP_OF[jb]
                    first = jb == T_STARTS[gg]
                    last = jb == T_STARTS[gg] + T_LIST[gg] - 1
                    nc.tensor.matmul(
                        out_ps[b][W:W + WW, :P + 1],
                        lhsT=Et[:, j * WW:(j + 1) * WW],
                        rhs=rhs[:, j, :P + 1],
                        start=first, stop=last,
                        tile_position=(0, W),
                        skip_group_check=True)
                    if jb == T_B - 1:
                        rec = wk.tile([P, 1], f32, tag="rec")
                        nc.vector.reciprocal(rec[:], out_ps[b][:, P:P + 1])
                        o = wk.tile([P, P], f32, tag="o")
                        nc.vector.tensor_scalar_mul(
                            out=o[:], in0=out_ps[b][:, :P], scalar1=rec[:])
                        nc.sync.dma_start(
                            out=d_out[b * P:(b + 1) * P, :], in_=o[:])
                        del out_ps[b]

    nc.compile()
    return nc


def _budgets(edge_index, n_nodes):
    """Per-window-group subtile budgets, maxed over cores/blocks."""
    G = P // WW
    r = np.asarray(edge_index[1], dtype=np.int64)
    npc = -(-n_nodes // (N_CORES * P)) * P
    while (npc * N_CORES) % 512:
        npc += P
    cnt = np.bincount(r // WW, minlength=(npc * N_CORES) // WW)
    return [max(1, int(-(-cnt[g::G].max() // P))) for g in range(G)]


def kernel(nodes, edges, edge_index, Wq, bq, Wk, bk, Wv, bv, **_unused):
    nodes = np.asarray(nodes)
    edges = np.asarray(edges)
    edge_index = np.asarray(edge_index)
    n_nodes, d_v = nodes.shape
    n_edges, d_e = edges.shape
    d_attn = Wq.shape[0]
    assert not np.any(bq) and not np.any(bk) and not np.any(bv), \
        "zero biases assumed"

    t_list = _budgets(edge_index, n_nodes)
    cfg = _cfg_from_shapes(n_nodes, n_edges, d_v, d_e, d_attn, t_list)

    in_maps = _host_prep(nodes, edges, edge_index,
                         np.asarray(Wq), np.asarray(Wk), np.asarray(Wv), cfg)
    nc = _build_program(cfg)

    from concourse.bass_utils import run_bass_kernel_spmd
    res = run_bass_kernel_spmd(nc, in_maps, core_ids=list(range(N_CORES)))
    out = np.concatenate([res.results[c]["out"] for c in range(N_CORES)], axis=0)
    return np.ascontiguousarray(out[:n_nodes]).astype(np.float32)
